# revision 1
# baseline (speedup 1.0000x reference)
"""Trainium2 Bass kernel for nn_ConsistentSelfAttentionProcessor.

Reference computation (per frame-set of NUM_FRAMES=4 frames):
    q,k,v = hs@Wq+bq, hs@Wk+bk, hs@Wv+bv          # [BF,S,D]
    per head: K_comb = [K(frame0_of_set); K(own)]  # 2S keys
    out = softmax(q@K_comb^T/sqrt(hd)) @ V_comb @ Wo + bo + hs

Sharding: 8 cores = 2 frame-sets x 4 head-groups of 5 heads.
Each core computes a partial output  attn(set, heads_g) @ Wo[rows_g]  in bf16;
the host sums the 4 per-set partials in fp32 and adds bo + residual.

Frame 0 of each set attends to [K0;K0] which equals softmax over K0 alone,
so frame 0 uses 1024 keys instead of 2048.

Softmax uses no max subtraction: scores*0.125 is bounded (~|3|) for these
inputs, so exp is safe in fp32. The softmax denominator comes for free from a
ones-column appended to V (U_T row 64 = sum(exp)).
"""

import sys
from contextlib import ExitStack

import numpy as np

sys.path.insert(0, "/opt/trn_rl_repo")

import ml_dtypes  # noqa: E402

import concourse.bass as bass  # noqa: E402
import concourse.mybir as mybir  # noqa: E402
import concourse.tile as tile  # noqa: E402
from concourse import bacc, bass_utils  # noqa: E402
from concourse.masks import make_identity  # noqa: E402

BF16 = mybir.dt.bfloat16
F32 = mybir.dt.float32
NPBF16 = ml_dtypes.bfloat16

NUM_FRAMES = 4
HEADS = 20
BF, S, D = 8, 1024, 1280
HD = 64  # head dim
B = BF // NUM_FRAMES  # 2 frame sets
N_CORES = 8
GROUPS = 4  # head groups per set
HG = HEADS // GROUPS  # 5 heads per group
C = HG * HD  # 320 columns per group
N_SET = NUM_FRAMES * S  # 4096 rows per set
SCALE = 1.0 / np.sqrt(HD)  # 0.125

P = 128
KC_D = D // P  # 10 contraction chunks for projections
TC_N = N_SET // P  # 32 token chunks per set
QH = 2  # q halves of 512 per frame


def build_kernel_body(ctx: ExitStack, tc: tile.TileContext, xt, wqkv, wo, bqkv, out):
    """Emit the per-core program.

    xt:   [D, N_SET]      bf16  (X^T for this set)
    wqkv: [D, 3*C]        bf16  (columns: Wq_g | Wk_g | Wv_g)
    wo:   [3*P, D]        bf16  (rows 0..C-1 = Wo[group rows]; rest zero pad)
    bqkv: [3*C]           f32
    out:  [N_SET, D]      bf16  (partial output, unsummed, no bo/residual)
    """
    nc = tc.nc

    const = ctx.enter_context(tc.tile_pool(name="const", bufs=1))
    persist = ctx.enter_context(tc.tile_pool(name="persist", bufs=1))
    work = ctx.enter_context(tc.tile_pool(name="work", bufs=3))
    psum = ctx.enter_context(tc.tile_pool(name="psum", bufs=1, space="PSUM"))

    # ---- constants ----------------------------------------------------------
    ident = const.tile([P, P], BF16, tag="ident")
    make_identity(nc, ident)
    ones = const.tile([P, P], F32, tag="ones")
    nc.gpsimd.memset(ones, 1.0)

    wqkv_sb = const.tile([P, KC_D, 3 * C], BF16, tag="wqkv")
    nc.sync.dma_start(wqkv_sb, wqkv.rearrange("(c p) n -> p c n", p=P))
    wo_sb = const.tile([P, 3, D], BF16, tag="wo")
    nc.sync.dma_start(wo_sb, wo.rearrange("(c p) n -> p c n", p=P))
    bqkv_sb = const.tile([1, 3 * C], F32, tag="bqkv")
    nc.sync.dma_start(bqkv_sb, bqkv[None, :])

    # broadcast biases across partitions once: bias_bc[p, j] = bqkv[j]
    bias_bc = const.tile([P, 3 * C], F32, tag="bias_bc")
    bps = psum.tile([P, 3 * C], F32, tag="A", bufs=2)
    nc.tensor.matmul(bps[:, 0:512], ones[0:1, :], bqkv_sb[:, 0:512])
    nc.tensor.matmul(bps[:, 512:960], ones[0:1, :], bqkv_sb[:, 512:960])
    nc.vector.tensor_copy(bias_bc, bps)

    # ---- persistent intermediates ------------------------------------------
    # Q^T/K^T, head-transposed: chunk h//2 holds head pair, base (h%2)*64.
    # chunks 0-2: q-heads, 3-5: k-heads (halves of chunks 2 and 5 unused).
    qkt = persist.tile([P, 6, N_SET], BF16, tag="qkt")
    # V rows with a ones column per head: [tokens, head, 65]
    vsb = persist.tile([P, TC_N, HG, HD + 1], BF16, tag="vsb")
    nc.gpsimd.memset(vsb[:, :, :, HD], 1.0)
    # attn^T for O-proj, one tensor per frame so O-proj(f) only depends on
    # frame f's attention: chunk c holds heads (2c, 2c+1); chunk 2 half unused
    atn_f = [
        persist.tile([P, 3, S], BF16, tag=f"atn{f}", name=f"atn{f}")
        for f in range(NUM_FRAMES)
    ]
    for f in range(NUM_FRAMES):
        nc.gpsimd.memset(atn_f[f][64:128, 2, :], 0.0)

    # ---- phase 1: QKV projections ------------------------------------------
    for t in range(TC_N):
        xcol = work.tile([P, KC_D, P], BF16, tag="xcol")
        nc.sync.dma_start(
            xcol, xt[:, t * P : (t + 1) * P].rearrange("(c p) n -> p c n", p=P)
        )
        pq = psum.tile([P, 3 * C], F32, tag="A", bufs=2)
        for kc in range(KC_D):
            st, sp = kc == 0, kc == KC_D - 1
            nc.tensor.matmul(
                pq[:, 0:512], xcol[:, kc], wqkv_sb[:, kc, 0:512], start=st, stop=sp
            )
            nc.tensor.matmul(
                pq[:, 512:960], xcol[:, kc], wqkv_sb[:, kc, 512:960], start=st, stop=sp
            )
        # V part: bias add + split per head into vsb
        nc.vector.tensor_tensor(
            vsb[:, t, :, 0:HD],
            pq[:, 2 * C : 3 * C].rearrange("p (h d) -> p h d", d=HD),
            bias_bc[:, 2 * C : 3 * C].rearrange("p (h d) -> p h d", d=HD),
            mybir.AluOpType.add,
        )
        # QK part: bias add + cast, then PE-transpose into qkt
        rows = work.tile([P, 2 * C], BF16, tag="rows")
        nc.vector.tensor_tensor(
            rows, pq[:, 0 : 2 * C], bias_bc[:, 0 : 2 * C], mybir.AluOpType.add
        )
        # 6 transposes: (q0q1)(q2q3)(q4)(k0k1)(k2k3)(k4)
        for ch in range(6):
            width = HD if ch in (2, 5) else P
            src = rows[:, ch * P : ch * P + width] if ch < 3 else rows[
                :, C + (ch - 3) * P : C + (ch - 3) * P + width
            ]
            tp = psum.tile([P, P], BF16, tag="C", bufs=2)
            nc.tensor.transpose(tp[0:width, :], src, ident)
            nc.vector.tensor_copy(qkt[0:width, ch, t * P : (t + 1) * P], tp[0:width, :])

    # ---- phase 2+3: attention, O-proj per frame -----------------------------
    for f in range(NUM_FRAMES):
        qoff = f * S
        nkc = 8 if f == 0 else 16  # frame 0: ref==own, dedup
        for h in range(HG):
            b = (h % 2) * HD  # partition base for this head
            qch = h // 2
            kch = 3 + h // 2
            ut = psum.tile([P, S], F32, tag="ut", bufs=1)
            for kc in range(nkc):
                # key token position: first 8 chunks ref frame, rest own frame
                ktok = kc * P if kc < 8 else qoff + (kc - 8) * P
                sc = psum.tile([P, S], F32, tag="A", bufs=2)
                for q in range(QH):
                    nc.tensor.matmul(
                        sc[:, q * 512 : (q + 1) * 512],
                        qkt[b : b + HD, kch, ktok : ktok + P],
                        qkt[b : b + HD, qch, qoff + q * 512 : qoff + (q + 1) * 512],
                    )
                ex = work.tile([P, S], BF16, tag="ex")
                nc.scalar.activation(
                    ex, sc, mybir.ActivationFunctionType.Exp, scale=SCALE
                )
                for q in range(QH):
                    nc.tensor.matmul(
                        ut[0 : HD + 1, q * 512 : (q + 1) * 512],
                        vsb[:, ktok // P, h, :],
                        ex[:, q * 512 : (q + 1) * 512],
                        start=(kc == 0),
                        stop=(kc == nkc - 1),
                    )
            # normalize: attn^T = ut[0:64] / bcast(ut[64]).  Keep PE out of
            # this tail: DVE copies s to SBUF, idle GpSimd broadcasts it
            # across partitions, DVE divides (single PSUM operand rule ok).
            rc = work.tile([HD + 1, S], F32, tag="rc", bufs=2)
            nc.vector.reciprocal(rc[HD : HD + 1, :], ut[HD : HD + 1, :])
            for q in range(QH):
                qs = slice(q * 512, (q + 1) * 512)
                bcp = psum.tile([HD, 512], F32, tag="C", bufs=2)
                nc.tensor.matmul(bcp, ones[HD : HD + 1, 0:HD], rc[HD : HD + 1, qs])
                # DVE can read only one PSUM operand per op: stage via SBUF
                bc = work.tile([HD, 512], F32, tag="bcs", bufs=2)
                nc.vector.tensor_copy(bc, bcp)
                if h % 2 == 0:
                    nc.vector.tensor_tensor(
                        atn_f[f][0:HD, h // 2, q * 512 : (q + 1) * 512],
                        ut[0:HD, qs],
                        bc,
                        mybir.AluOpType.mult,
                    )
                else:
                    # result must land at partitions 64-127: mult to a base-0
                    # tmp, then PE-copy shifts partitions
                    tm = work.tile([HD, 512], BF16, tag="tm", bufs=2)
                    nc.vector.tensor_tensor(tm, ut[0:HD, qs], bc, mybir.AluOpType.mult)
                    pc = psum.tile([P, 512], F32, tag="C", bufs=2)
                    nc.tensor.matmul(pc[HD:P, :], ident[0:HD, 0:HD], tm)
                    nc.vector.tensor_copy(
                        atn_f[f][HD:P, h // 2, q * 512 : (q + 1) * 512],
                        pc[HD:P, :],
                    )
        # O-proj for this frame's 8 token chunks (fills ACT-bound gaps of the
        # next frame's attention on PE)
        for tl in range(S // P):
            t = f * (S // P) + tl
            ou = work.tile([P, D], BF16, tag="ou")
            for n3, nw in ((0, 512), (1, 512), (2, 256)):
                po = psum.tile([P, 512], F32, tag="A", bufs=2)
                for kc in range(3):
                    nc.tensor.matmul(
                        po[:, 0:nw],
                        atn_f[f][:, kc, tl * P : (tl + 1) * P],
                        wo_sb[:, kc, n3 * 512 : n3 * 512 + nw],
                        start=(kc == 0),
                        stop=(kc == 2),
                    )
                nc.vector.tensor_copy(ou[:, n3 * 512 : n3 * 512 + nw], po[:, 0:nw])
            nc.sync.dma_start(out[t * P : (t + 1) * P, :], ou)


def build_program():
    from concourse.bass_interp import get_hw_module

    nc = bacc.Bacc(
        "TRN2",
        target_bir_lowering=False,
        debug=False,
        enable_asserts=False,
        num_devices=N_CORES,
    )
    xt = nc.dram_tensor("xt", [D, N_SET], BF16, kind="ExternalInput").ap()
    wqkv = nc.dram_tensor("wqkv", [D, 3 * C], BF16, kind="ExternalInput").ap()
    wo = nc.dram_tensor("wo", [3 * P, D], BF16, kind="ExternalInput").ap()
    bqkv = nc.dram_tensor("bqkv", [3 * C], F32, kind="ExternalInput").ap()
    out = nc.dram_tensor("out", [N_SET, D], BF16, kind="ExternalOutput").ap()
    with tile.TileContext(nc) as tc:
        with ExitStack() as ctx:
            build_kernel_body(ctx, tc, xt, wqkv, wo, bqkv, out)
    nc.finalize()
    nc.m = get_hw_module(nc.m)
    return nc


def make_in_maps(hidden_states, Wq, Wk, Wv, bq, bk, bv):
    """Per-core inputs. Core c = set (c//4), head group (c%4)."""
    hs = np.asarray(hidden_states, np.float32).reshape(BF, S, D)
    in_maps = []
    xts = []
    for s in range(B):
        x = hs[s * NUM_FRAMES : (s + 1) * NUM_FRAMES].reshape(N_SET, D)
        xts.append(np.ascontiguousarray(x.T).astype(NPBF16))
    for c in range(N_CORES):
        s, g = c // GROUPS, c % GROUPS
        cols = slice(g * C, (g + 1) * C)
        wqkv = np.concatenate(
            [np.asarray(W, np.float32)[:, cols] for W in (Wq, Wk, Wv)], axis=1
        ).astype(NPBF16)
        bqkv = np.concatenate(
            [np.asarray(bb, np.float32)[cols] for bb in (bq, bk, bv)]
        ).astype(np.float32)
        in_maps.append(
            {"xt": xts[s], "wqkv": wqkv, "bqkv": bqkv}
        )
    return in_maps


def make_wo_pad(Wo, g):
    wo_g = np.asarray(Wo, np.float32)[g * C : (g + 1) * C, :]  # [320, 1280]
    wo_pad = np.zeros((3 * P, D), np.float32)
    wo_pad[:C] = wo_g
    return wo_pad.astype(NPBF16)


_PROGRAM = None


def kernel(hidden_states, Wq, Wk, Wv, Wo, bq, bk, bv, bo):
    global _PROGRAM
    if _PROGRAM is None:
        _PROGRAM = build_program()
    nc = _PROGRAM

    in_maps = make_in_maps(hidden_states, Wq, Wk, Wv, bq, bk, bv)
    for c in range(N_CORES):
        in_maps[c]["wo"] = make_wo_pad(Wo, c % GROUPS)

    res = bass_utils.run_bass_kernel_spmd(nc, in_maps, core_ids=list(range(N_CORES)))
    hs = np.asarray(hidden_states, np.float32)
    bo = np.asarray(bo, np.float32)
    out = np.empty((BF, S, D), np.float32)
    for s in range(B):
        acc = np.zeros((N_SET, D), np.float32)
        for g in range(GROUPS):
            acc += np.asarray(res.results[s * GROUPS + g]["out"], np.float32)
        out[s * NUM_FRAMES : (s + 1) * NUM_FRAMES] = (
            acc.reshape(NUM_FRAMES, S, D)
            + bo[None, None, :]
            + hs[s * NUM_FRAMES : (s + 1) * NUM_FRAMES]
        )
    return out



# revision 2
# speedup vs baseline: 1.2527x; 1.2527x over previous
"""Trainium2 Bass kernel for nn_ConsistentSelfAttentionProcessor.

Reference computation (per frame-set of NUM_FRAMES=4 frames):
    q,k,v = hs@Wq+bq, hs@Wk+bk, hs@Wv+bv          # [BF,S,D]
    per head: K_comb = [K(frame0_of_set); K(own)]  # 2S keys
    out = softmax(q@K_comb^T/sqrt(hd)) @ V_comb @ Wo + bo + hs

Sharding: 8 cores = 2 frame-sets x 4 head-groups of 5 heads.
Each core computes a partial output  attn(set, heads_g) @ Wo[rows_g]  in bf16;
the host sums the 4 per-set partials in fp32 and adds bo + residual.

v1 design notes:
- Q/K projections run weight-stationary so the PSUM output is already
  head-transposed ([outcol, token]) -- no PE transposes, no DVE copies.
  Host reorders W columns into 6 chunks of 128: [Q01|Q23|Q4z|K01|K23|K4z]
  so each head's Q and K share an intra-chunk partition offset (h%2)*64.
- V projection stays X-stationary ([token, col]) since AV needs
  token-major lhsT.
- All per-frame intermediates (qkt/vsb/atn) are per-frame tensors so the
  scheduler can overlap frame f attention with frame f+1 projection and
  frame f-1 O-projection; emission interleaves those streams.
- Softmax tail is decoupled: ut (PSUM) is copied to SBUF immediately so
  the next head's AV can reuse the bank; reciprocal (DVE, slow on 1
  partition) + GpSimd partition_broadcast + DVE multiply all run off the
  PE critical path.  Odd heads' results reach partitions 64-127 via a
  SBUF->SBUF DMA (engines are lane-locked).
- Frame 0 of each set attends to [K0;K0] == softmax over K0 alone, so
  frame 0 uses 1024 keys instead of 2048.
- Softmax uses no max subtraction: scores*0.125 is bounded (~|3|) for
  these inputs, so exp is safe in fp32.  The denominator comes free from
  a ones-column appended to V (ut row 64 = sum(exp)).
"""

import sys
from contextlib import ExitStack

import numpy as np

sys.path.insert(0, "/opt/trn_rl_repo")

import ml_dtypes  # noqa: E402

import concourse.bass as bass  # noqa: E402
import concourse.mybir as mybir  # noqa: E402
import concourse.tile as tile  # noqa: E402
from concourse import bacc, bass_utils  # noqa: E402

BF16 = mybir.dt.bfloat16
F32 = mybir.dt.float32
NPBF16 = ml_dtypes.bfloat16

NUM_FRAMES = 4
HEADS = 20
BF, S, D = 8, 1024, 1280
HD = 64  # head dim
B = BF // NUM_FRAMES  # 2 frame sets
N_CORES = 8
GROUPS = 4  # head groups per set
HG = HEADS // GROUPS  # 5 heads per group
C = HG * HD  # 320 columns per group
N_SET = NUM_FRAMES * S  # 4096 rows per set
SCALE = 1.0 / np.sqrt(HD)  # 0.125
QKP = 384  # padded Q (and K) column block: 5 heads * 64 + 64 zero pad
WCOLS = 2 * QKP + C  # 1088 host-side wqkv columns

P = 128
KC_D = D // P  # 10 contraction chunks for projections
TPF = S // P  # 8 token chunks per frame
QH = 2  # q halves of 512 per frame


def build_kernel_body(ctx: ExitStack, tc: tile.TileContext, xt, wqkv, wo, bqk, bv, out):
    """Emit the per-core program.

    xt:   [D, N_SET]      bf16  (X^T for this set)
    wqkv: [D, 1088]       bf16  (columns: Q01|Q23|Q4z|K01|K23|K4z|V, z=64 zero)
    wo:   [3*P, D]        bf16  (rows 0..C-1 = Wo[group rows]; rest zero pad)
    bqk:  [768]           f32   (same column order as wqkv's first 768)
    bv:   [320]           f32
    out:  [N_SET, D]      bf16  (partial output, unsummed, no bo/residual)
    """
    nc = tc.nc

    const = ctx.enter_context(tc.tile_pool(name="const", bufs=1))
    persist = ctx.enter_context(tc.tile_pool(name="persist", bufs=1))
    work = ctx.enter_context(tc.tile_pool(name="work", bufs=2))
    psum = ctx.enter_context(tc.tile_pool(name="psum", bufs=1, space="PSUM"))

    # ---- constants ----------------------------------------------------------
    ones = const.tile([1, P], F32, tag="ones")
    nc.gpsimd.memset(ones, 1.0)

    wqkv_sb = const.tile([P, KC_D, WCOLS], BF16, tag="wqkv")
    nc.sync.dma_start(wqkv_sb, wqkv.rearrange("(c p) n -> p c n", p=P))
    wo_sb = const.tile([P, 3, D], BF16, tag="wo")
    nc.sync.dma_start(wo_sb, wo.rearrange("(c p) n -> p c n", p=P))
    bqk_sb = const.tile([P, 6], F32, tag="bqk")
    nc.sync.dma_start(bqk_sb, bqk.rearrange("(c p) -> p c", p=P))
    bv_sb = const.tile([1, C], F32, tag="bv")
    nc.sync.dma_start(bv_sb, bv[None, :])

    # broadcast V bias across partitions once: bias_v[p, j] = bv[j]
    bias_v = const.tile([P, C], F32, tag="bias_v")
    bps = psum.tile([P, C], F32, tag="A", bufs=2)
    nc.tensor.matmul(bps, ones[0:1, :], bv_sb)
    nc.vector.tensor_copy(bias_v, bps)

    # ---- persistent per-frame intermediates ---------------------------------
    # Q^T/K^T, head-transposed: chunk h//2 holds Q head pair at partition
    # base (h%2)*64; chunk 3+h//2 holds the matching K pair.  Upper halves
    # of chunks 2 and 5 are zero pad (written, never read).
    qkt_f = [
        persist.tile([P, 6, S], BF16, tag=f"qkt{f}", name=f"qkt{f}")
        for f in range(NUM_FRAMES)
    ]
    # V rows with a ones column per head: [tokens, chunk, head, 65]
    vsb_f = [
        persist.tile([P, TPF, HG, HD + 1], BF16, tag=f"vsb{f}", name=f"vsb{f}")
        for f in range(NUM_FRAMES)
    ]
    for f in range(NUM_FRAMES):
        nc.gpsimd.memset(vsb_f[f][:, :, :, HD], 1.0)
    # attn^T for O-proj: chunk c holds heads (2c, 2c+1); chunk 2 half unused
    atn_f = [
        persist.tile([P, 3, S], BF16, tag=f"atn{f}", name=f"atn{f}")
        for f in range(NUM_FRAMES)
    ]
    for f in range(NUM_FRAMES):
        nc.gpsimd.memset(atn_f[f][HD:P, 2, :], 0.0)

    # ---- generators ---------------------------------------------------------

    def gen_proj_dma(f):
        xsb = work.tile([P, KC_D, S], BF16, tag="xsb", bufs=2, name=f"xsb{f}")
        for tg in range(QH):
            nc.sync.dma_start(
                xsb[:, :, tg * 512 : (tg + 1) * 512],
                xt[:, f * S + tg * 512 : f * S + (tg + 1) * 512].rearrange(
                    "(c p) n -> p c n", p=P
                ),
            )
        return xsb

    def gen_proj_qk(f, xsb, och):
        # weight-stationary: psum[outcol, tok] accumulated over d-chunks
        for tg in range(QH):
            pqk = psum.tile([P, 512], F32, tag="A", bufs=2)
            for kc in range(KC_D):
                nc.tensor.matmul(
                    pqk,
                    wqkv_sb[:, kc, och * P : (och + 1) * P],
                    xsb[:, kc, tg * 512 : (tg + 1) * 512],
                    start=(kc == 0),
                    stop=(kc == KC_D - 1),
                )
            # bias add (per-partition scalar) + bf16 cast on the way out
            nc.vector.tensor_scalar(
                qkt_f[f][:, och, tg * 512 : (tg + 1) * 512],
                pqk,
                bqk_sb[:, och : och + 1],
                None,
                mybir.AluOpType.add,
            )

    def gen_proj_v(f, xsb, tl):
        # X-stationary: psum[tok, vcol]
        pv = psum.tile([P, C], F32, tag="A", bufs=2)
        for kc in range(KC_D):
            nc.tensor.matmul(
                pv,
                xsb[:, kc, tl * P : (tl + 1) * P],
                wqkv_sb[:, kc, 2 * QKP : WCOLS],
                start=(kc == 0),
                stop=(kc == KC_D - 1),
            )
        nc.vector.tensor_tensor(
            vsb_f[f][:, tl, :, 0:HD],
            pv.rearrange("p (h d) -> p h d", d=HD),
            bias_v.rearrange("p (h d) -> p h d", d=HD),
            mybir.AluOpType.add,
        )

    def gen_attn(f, h):
        b = (h % 2) * HD
        qch = h // 2
        kch = 3 + h // 2
        nkc = TPF if f == 0 else 2 * TPF  # frame 0: ref==own, dedup
        ut = psum.tile([HD + 1, S], F32, tag="ut", bufs=1)
        for kc in range(nkc):
            fk, tlk = (0, kc) if kc < TPF else (f, kc - TPF)
            kt = tlk * P
            sc = psum.tile([P, S], F32, tag="S", bufs=2)
            for q in range(QH):
                nc.tensor.matmul(
                    sc[:, q * 512 : (q + 1) * 512],
                    qkt_f[fk][b : b + HD, kch, kt : kt + P],
                    qkt_f[f][b : b + HD, qch, q * 512 : (q + 1) * 512],
                )
            ex = work.tile([P, S], BF16, tag="ex", bufs=3)
            nc.scalar.activation(ex, sc, mybir.ActivationFunctionType.Exp, scale=SCALE)
            for q in range(QH):
                nc.tensor.matmul(
                    ut[:, q * 512 : (q + 1) * 512],
                    vsb_f[fk][:, tlk, h, :],
                    ex[:, q * 512 : (q + 1) * 512],
                    start=(kc == 0),
                    stop=(kc == nkc - 1),
                )
        # decouple: copy ut out of PSUM fast, then normalize from SBUF
        usb = work.tile([HD + 1, S], F32, tag="usb", bufs=2)
        nc.vector.tensor_copy(usb, ut)
        rc = work.tile([1, S], F32, tag="rc", bufs=2)
        nc.vector.reciprocal(rc, usb[HD : HD + 1, :])
        rcb = work.tile([HD, S], F32, tag="rcb", bufs=2)
        nc.gpsimd.partition_broadcast(rcb, rc)
        if h % 2 == 0:
            nc.vector.tensor_tensor(
                atn_f[f][0:HD, qch, :], usb[0:HD, :], rcb, mybir.AluOpType.mult
            )
        else:
            # result must land at partitions 64-127: engines are lane-locked,
            # so multiply at base 0 and partition-shift via SBUF->SBUF DMA
            tm = work.tile([HD, S], BF16, tag="tm", bufs=2)
            nc.vector.tensor_tensor(tm, usb[0:HD, :], rcb, mybir.AluOpType.mult)
            nc.sync.dma_start(atn_f[f][HD:P, qch, :], tm)

    def gen_oproj(f, tl):
        t = f * TPF + tl
        ou = work.tile([P, D], BF16, tag="ou", bufs=2)
        for n3, nw in ((0, 512), (1, 512), (2, 256)):
            po = psum.tile([P, 512], F32, tag="A", bufs=2)
            for kc in range(3):
                nc.tensor.matmul(
                    po[:, 0:nw],
                    atn_f[f][:, kc, tl * P : (tl + 1) * P],
                    wo_sb[:, kc, n3 * 512 : n3 * 512 + nw],
                    start=(kc == 0),
                    stop=(kc == 2),
                )
            nc.vector.tensor_copy(ou[:, n3 * 512 : n3 * 512 + nw], po[:, 0:nw])
        nc.sync.dma_start(out[t * P : (t + 1) * P, :], ou)

    # ---- emission: frame 0 projection, then per-frame attention with
    # next-frame projection and prev-frame O-proj interleaved ----------------
    def proj_units(f):
        xsb = gen_proj_dma(f)
        units = [lambda och=och: gen_proj_qk(f, xsb, och) for och in range(6)]
        units += [lambda tl=tl: gen_proj_v(f, xsb, tl) for tl in range(TPF)]
        return units

    for u in proj_units(0):
        u()
    for f in range(NUM_FRAMES):
        pu = proj_units(f + 1) if f < NUM_FRAMES - 1 else []
        ou_q = [(f - 1, tl) for tl in range(TPF)] if f > 0 else []
        for h in range(HG):
            gen_attn(f, h)
            for _ in range(3):
                if pu:
                    pu.pop(0)()
            for _ in range(2):
                if ou_q:
                    gen_oproj(*ou_q.pop(0))
        while pu:
            pu.pop(0)()
        while ou_q:
            gen_oproj(*ou_q.pop(0))
    for tl in range(TPF):
        gen_oproj(NUM_FRAMES - 1, tl)


def build_program():
    from concourse.bass_interp import get_hw_module

    nc = bacc.Bacc(
        "TRN2",
        target_bir_lowering=False,
        debug=False,
        enable_asserts=False,
        num_devices=N_CORES,
    )
    xt = nc.dram_tensor("xt", [D, N_SET], BF16, kind="ExternalInput").ap()
    wqkv = nc.dram_tensor("wqkv", [D, WCOLS], BF16, kind="ExternalInput").ap()
    wo = nc.dram_tensor("wo", [3 * P, D], BF16, kind="ExternalInput").ap()
    bqk = nc.dram_tensor("bqk", [6 * P], F32, kind="ExternalInput").ap()
    bv = nc.dram_tensor("bv", [C], F32, kind="ExternalInput").ap()
    out = nc.dram_tensor("out", [N_SET, D], BF16, kind="ExternalOutput").ap()
    with tile.TileContext(nc) as tc:
        with ExitStack() as ctx:
            build_kernel_body(ctx, tc, xt, wqkv, wo, bqk, bv, out)
    nc.finalize()
    nc.m = get_hw_module(nc.m)
    return nc


def make_in_maps(hidden_states, Wq, Wk, Wv, bq, bk, bv):
    """Per-core inputs. Core c = set (c//4), head group (c%4)."""
    hs = np.asarray(hidden_states, np.float32).reshape(BF, S, D)
    in_maps = []
    xts = []
    for s in range(B):
        x = hs[s * NUM_FRAMES : (s + 1) * NUM_FRAMES].reshape(N_SET, D)
        xts.append(np.ascontiguousarray(x.T).astype(NPBF16))
    for c in range(N_CORES):
        s, g = c // GROUPS, c % GROUPS
        cols = slice(g * C, (g + 1) * C)
        wq_g = np.asarray(Wq, np.float32)[:, cols]
        wk_g = np.asarray(Wk, np.float32)[:, cols]
        wv_g = np.asarray(Wv, np.float32)[:, cols]
        z = np.zeros((D, QKP - C), np.float32)
        wqkv = np.concatenate([wq_g, z, wk_g, z, wv_g], axis=1).astype(NPBF16)
        bq_g = np.asarray(bq, np.float32)[cols]
        bk_g = np.asarray(bk, np.float32)[cols]
        zb = np.zeros(QKP - C, np.float32)
        bqk = np.concatenate([bq_g, zb, bk_g, zb]).astype(np.float32)
        bv_g = np.asarray(bv, np.float32)[cols].astype(np.float32)
        in_maps.append({"xt": xts[s], "wqkv": wqkv, "bqk": bqk, "bv": bv_g})
    return in_maps


def make_wo_pad(Wo, g):
    wo_g = np.asarray(Wo, np.float32)[g * C : (g + 1) * C, :]  # [320, 1280]
    wo_pad = np.zeros((3 * P, D), np.float32)
    wo_pad[:C] = wo_g
    return wo_pad.astype(NPBF16)


_PROGRAM = None


def kernel(hidden_states, Wq, Wk, Wv, Wo, bq, bk, bv, bo):
    global _PROGRAM
    if _PROGRAM is None:
        _PROGRAM = build_program()
    nc = _PROGRAM

    in_maps = make_in_maps(hidden_states, Wq, Wk, Wv, bq, bk, bv)
    for c in range(N_CORES):
        in_maps[c]["wo"] = make_wo_pad(Wo, c % GROUPS)

    res = bass_utils.run_bass_kernel_spmd(nc, in_maps, core_ids=list(range(N_CORES)))
    hs = np.asarray(hidden_states, np.float32)
    bo = np.asarray(bo, np.float32)
    out = np.empty((BF, S, D), np.float32)
    for s in range(B):
        acc = np.zeros((N_SET, D), np.float32)
        for g in range(GROUPS):
            acc += np.asarray(res.results[s * GROUPS + g]["out"], np.float32)
        out[s * NUM_FRAMES : (s + 1) * NUM_FRAMES] = (
            acc.reshape(NUM_FRAMES, S, D)
            + bo[None, None, :]
            + hs[s * NUM_FRAMES : (s + 1) * NUM_FRAMES]
        )
    return out
